# revision 5
# baseline (speedup 1.0000x reference)
"""Trainium2 Bass kernel for multi-head attention (b=4, n=2048, d=512, h=8, dk=dv=64).

Sharding: 8 cores = 4 batches x 2 query-halves. Each core computes K/V for its
full batch sequence (2048) and attention outputs for its 1024 query rows.
No collectives needed; host stacks the per-core [1024, 512] outputs.

Per-core dataflow (f32r = TF32-like fast fp32 matmul mode; PV in bf16):
  x^T [512, 2048] staged in SBUF; all projections run up front, ordered to
  overlap the HBM input stream (wv, x quarters, wq, wk, wo).
  Q/K projections are head-PAIR packed: one [128 = h_even dims | h_odd dims]
  PSUM tile per pair covers two heads per moving stream (halved MM columns,
  unreplicated wq/wk).  Per-head S^T matmuls then use 64-partition operands
  at base partition (h%2)*64 (PE quadrant placement via tile_position).
  S^T chunks [128 j, 1024 i] are DVE-cast to fp16 in SBUF, exp'd by ONE
  scalar ACT per 8 chunks ([128, 8192], amortizing the 352-cyc ACT startup),
  output bf16.  PV accumulates [65, 1024] per head over 16 j-chunks with the
  ones-column denominator trick; out^T scaled by 1/denom; y = out^T.T @ Wo + bo.
"""
import numpy as np

B, N, MODEL = 4, 2048, 512
H, DK = 8, 64
SCALE = DK ** -0.5
NI = 1024           # query rows per core
NCH = MODEL // 128  # model-dim chunks
NJC = N // 128      # key/value chunks
NHP = H // 2        # head pairs
ABATCH = 8          # j-chunks per exp ACT batch

_COMPILED = None


def _build():
    import concourse.bass as bass
    from concourse import bacc
    import concourse.mybir as mybir
    import concourse.tile as tile

    F32 = mybir.dt.float32
    F32R = mybir.dt.float32r
    BF16 = mybir.dt.bfloat16
    F16 = mybir.dt.float16
    EXP = mybir.ActivationFunctionType.Exp

    nc = bacc.Bacc("TRN2", target_bir_lowering=False, debug=False, num_devices=8)
    xt_in = nc.dram_tensor("xt", [MODEL, N], F32R, kind="ExternalInput")
    wq_in = nc.dram_tensor("wq", [MODEL, MODEL], F32R, kind="ExternalInput")
    wk_in = nc.dram_tensor("wk", [MODEL, MODEL], F32R, kind="ExternalInput")
    wv_in = nc.dram_tensor("wv", [MODEL, MODEL], F32R, kind="ExternalInput")
    relb_in = nc.dram_tensor("relb", [128, NHP], F32, kind="ExternalInput")
    wo_in = nc.dram_tensor("wo", [MODEL, MODEL], F32R, kind="ExternalInput")
    bo_in = nc.dram_tensor("bo", [1, MODEL], F32, kind="ExternalInput")
    onesb_in = nc.dram_tensor("onesb", [128, NJC * H], BF16, kind="ExternalInput")
    y_out = nc.dram_tensor("y", [NI, MODEL], F32, kind="ExternalOutput")

    with tile.TileContext(nc) as tc:
        with (
            tc.tile_pool(name="w", bufs=1) as wp,
            tc.tile_pool(name="acts", bufs=1) as ap,
            tc.tile_pool(name="big", bufs=2, space="PSUM") as ps,
        ):
            # ---------- persistent tiles ----------
            wo = wp.tile([128, NCH, MODEL], F32R, tag="wo")
            bo = wp.tile([1, MODEL], F32, tag="bo")
            bo_b = wp.tile([128, MODEL], F32, tag="bo_b")
            vv_a = ap.tile([128, NJC // 2, H * 65], BF16, tag="vva")
            vv_b = ap.tile([128, NJC // 2, H * 65], BF16, tag="vvb")
            def vvt(jc):
                return (vv_a if jc < NJC // 2 else vv_b)[:, jc % (NJC // 2)]
            relb = ap.tile([128, NHP], F32, tag="relb")
            outt = ap.tile([128, NCH, NI], F32R, tag="outt")
            kt = ap.tile([128, NHP, NJC, 128], F32R, tag="kt")
            qt = ap.tile([128, NHP, NI], F32R, tag="qt")

            def r3(d):
                return d[:].rearrange("(c p) n -> p c n", p=128)

            dma_engs = [None]
            def dma(out, in_):
                dma_engs[0] = (dma_engs[0] or 0) + 1
                engs = (nc.sync, nc.gpsimd, nc.scalar)
                engs[dma_engs[0] % 3].dma_start(out=out, in_=in_)

            with tc.tile_pool(name="proj", bufs=1) as pp, \
                 tc.tile_pool(name="qk", bufs=2, space="PSUM") as qkp:
                xt0 = pp.tile([128, NCH, 512], F32R, tag="xt0")
                xt1 = pp.tile([128, NCH, 512], F32R, tag="xt1")
                xt2 = pp.tile([128, NCH, 512], F32R, tag="xt2")
                xt3 = pp.tile([128, NCH, 512], F32R, tag="xt3")
                xts = [xt0, xt1, xt2, xt3]
                wq = pp.tile([128, NCH, MODEL], F32R, tag="wq")
                wk = pp.tile([128, NCH, MODEL], F32R, tag="wk")
                wv = pp.tile([128, NCH, MODEL], F32R, tag="wv")
                onesb_t = pp.tile([128, NJC * H], BF16, tag="onesb")

                # ---- DMA emission in global priority order (round-robin) ----
                xsrc = r3(xt_in)
                def dma_x(q):
                    for chh in range(2):
                        dma(xts[q][:, chh * 2:(chh + 1) * 2, :],
                            xsrc[:, chh * 2:(chh + 1) * 2, q * 512:(q + 1) * 512])
                for ch in range(NCH):
                    dma(wv[:, ch], r3(wv_in)[:, ch])
                dma_x(0)
                for ch in range(NCH):
                    dma(wq[:, ch], r3(wq_in)[:, ch])
                dma_x(1)
                dma_x(2)
                for ch in range(NCH):
                    dma(wk[:, ch], r3(wk_in)[:, ch])
                dma_x(3)
                for ch in range(NCH):
                    dma(wo[:, ch], r3(wo_in)[:, ch])
                dma(bo[:], bo_in[:])
                dma(relb[:], relb_in[:])
                dma(onesb_t[:], onesb_in[:])
                nc.gpsimd.partition_broadcast(bo_b[:], bo[:])
                # ones columns of V_aug: contiguous DMA to scratch, strided copy
                for vh in range(2):
                    nc.vector.tensor_copy(
                        (vv_a if vh == 0 else vv_b)[:]
                        .rearrange("p j (h e) -> p (j h) e", e=65)[:, :, 64:65],
                        onesb_t[:, vh * NJC * H // 2:(vh + 1) * NJC * H // 2]
                        .rearrange("p (n o) -> p n o", o=1))

                def xtv(ch, start, size):
                    t = xts[start // 512]
                    off = start % 512
                    assert off + size <= 512
                    return t[:, ch, off:off + size]

                # ---- compute emission, ordered to match DMA arrival ----
                def emit_v(jcs):
                    for jc in jcs:
                        v_ps = ps.tile([128, NI], F32, tag="big")
                        for ch in range(NCH):
                            nc.tensor.matmul(v_ps[:, 0:MODEL],
                                             xtv(ch, jc * 128, 128),
                                             wv[:, ch],
                                             start=(ch == 0), stop=(ch == NCH - 1))
                        nc.vector.tensor_copy(
                            vvt(jc).rearrange("p (h e) -> p h e", e=65)[:, :, 0:64],
                            v_ps[:, 0:MODEL].rearrange("p (h e) -> p h e", e=64))

                def emit_q(hp):
                    q_ps = qkp.tile([128, NI], F32, tag="qk")
                    for ib in range(2):
                        for ch in range(NCH):
                            nc.tensor.matmul(
                                q_ps[:, ib * 512:(ib + 1) * 512],
                                wq[:, ch, hp * 128:(hp + 1) * 128],
                                xtv(ch, ib * 512, 512),
                                start=(ch == 0), stop=(ch == NCH - 1))
                    nc.vector.tensor_scalar_add(qt[:, hp], q_ps[:],
                                                relb[:, hp:hp + 1])

                def emit_k(hp, jb):
                    k_ps = qkp.tile([128, NI], F32, tag="qk")
                    for sb in range(2):
                        off = jb * NI + sb * 512
                        for ch in range(NCH):
                            nc.tensor.matmul(
                                k_ps[:, sb * 512:(sb + 1) * 512],
                                wk[:, ch, hp * 128:(hp + 1) * 128],
                                xtv(ch, off, 512),
                                start=(ch == 0), stop=(ch == NCH - 1))
                    jcs = slice(jb * 8, jb * 8 + 8)
                    src = k_ps[:].rearrange("p (j m) -> p j m", m=128)
                    nc.vector.tensor_copy(kt[0:64, hp, jcs, :], src[0:64])
                    nc.vector.tensor_copy(kt[64:128, hp, jcs, :], src[64:128])

                emit_v(range(0, 4))
                emit_q(0)
                emit_q(1)
                emit_v(range(4, 8))
                emit_q(2)
                emit_q(3)
                emit_v(range(8, 12))
                for hp in range(NHP):
                    emit_k(hp, 0)
                emit_v(range(12, 16))
                for hp in range(NHP):
                    emit_k(hp, 1)

            # ---------- attention: per head S^T -> exp -> PV ----------
            with tc.tile_pool(name="pt", bufs=2) as ptp, \
                 tc.tile_pool(name="pv", bufs=2, space="PSUM") as pvp, \
                 tc.tile_pool(name="norm", bufs=2) as np_, \
                 tc.tile_pool(name="ysb", bufs=2) as yp_sb:
                for h in range(H):
                    hp, base = h // 2, (h % 2) * 64
                    pv_t = pvp.tile([65, NI], F32, tag="pv")
                    for ab in range(NJC // ABATCH):
                        p16 = ptp.tile([128, ABATCH * NI], F16, tag="p16")
                        pbf = ptp.tile([128, ABATCH * NI], BF16, tag="pbf")
                        for k in range(ABATCH):
                            jc = ab * ABATCH + k
                            st = ps.tile([128, NI], F32, tag="big")
                            for ih in range(2):
                                nc.tensor.matmul(
                                    st[:, ih * 512:(ih + 1) * 512],
                                    kt[base:base + 64, hp, jc],
                                    qt[base:base + 64, hp,
                                       ih * 512:(ih + 1) * 512],
                                    start=True, stop=True)
                            nc.vector.tensor_copy(
                                p16[:, k * NI:(k + 1) * NI], st[:])
                        nc.scalar.activation(pbf[:], p16[:], EXP, scale=1.0)
                        for k in range(ABATCH):
                            jc = ab * ABATCH + k
                            for ih in range(2):
                                nc.tensor.matmul(
                                    pv_t[:, ih * 512:(ih + 1) * 512],
                                    vvt(jc)[:, h * 65:(h + 1) * 65],
                                    pbf[:, k * NI + ih * 512:
                                        k * NI + (ih + 1) * 512],
                                    start=(jc == 0), stop=(jc == NJC - 1))
                    den = np_.tile([1, NI], F32, tag="den")
                    nc.vector.tensor_copy(den[:], pv_t[64:65, :])
                    rrow = np_.tile([1, NI], F32, tag="rrow")
                    nc.vector.reciprocal_approx_fast(rrow[:], den[:])
                    rb = np_.tile([64, NI], F32, tag="rb")
                    nc.gpsimd.partition_broadcast(rb[:], rrow[:])
                    nc.vector.tensor_tensor(
                        out=outt[base:base + 64, hp, :],
                        in0=pv_t[0:64, :], in1=rb[:],
                        op=mybir.AluOpType.mult)

                # ---------- output projection ----------
                for ib in range(NI // 128):
                    y_ps = ps.tile([128, NI], F32, tag="big")
                    for ch in range(NCH):
                        nc.tensor.matmul(y_ps[:, 0:MODEL],
                                         outt[:, ch, ib * 128:(ib + 1) * 128],
                                         wo[:, ch],
                                         start=(ch == 0), stop=(ch == NCH - 1))
                    y_sb = yp_sb.tile([128, MODEL], F32, tag="ysb")
                    nc.vector.tensor_tensor(out=y_sb[:], in0=y_ps[:, 0:MODEL],
                                            in1=bo_b[:],
                                            op=mybir.AluOpType.add)
                    dma(y_out[ib * 128:(ib + 1) * 128, :], y_sb[:])

    nc.compile()
    return nc


def _get_compiled():
    global _COMPILED
    if _COMPILED is None:
        _COMPILED = _build()
    return _COMPILED


def kernel(x, Wq, Wk, Wv, Wo, bo, rel_content_bias, _trace=False):
    from concourse.bass_utils import run_bass_kernel_spmd
    import ml_dtypes

    nc = _get_compiled()

    x = np.asarray(x, dtype=np.float32)
    Wq = np.asarray(Wq, dtype=np.float32)
    Wk = np.asarray(Wk, dtype=np.float32)
    Wv = np.asarray(Wv, dtype=np.float32)
    Wo = np.asarray(Wo, dtype=np.float32)
    bo = np.asarray(bo, dtype=np.float32)
    bias = np.asarray(rel_content_bias, dtype=np.float32).reshape(H, DK)

    Wq_s = (Wq * SCALE).astype(np.float32)
    # relb packed per head pair: rows 0:64 = even head bias, 64:128 = odd head
    relb = np.ascontiguousarray(
        bias.reshape(NHP, 2, DK).transpose(1, 2, 0).reshape(128, NHP))
    onesb = np.ones((128, NJC * H), ml_dtypes.bfloat16)
    shared = {"wq": Wq_s, "wk": Wk, "wv": Wv, "relb": relb, "wo": Wo,
              "bo": bo[None, :], "onesb": onesb}

    in_maps = []
    for c in range(8):
        b, half = c // 2, c % 2
        xt = np.ascontiguousarray(x[b].T)              # [512, 2048]
        if half:
            xt = np.ascontiguousarray(np.roll(xt, -NI, axis=1))
        in_maps.append({"xt": xt, **shared})

    res = run_bass_kernel_spmd(nc, in_maps, core_ids=list(range(8)),
                               trace=_trace)
    out = np.empty((B, N, MODEL), np.float32)
    for c in range(8):
        b, half = c // 2, c % 2
        out[b, half * NI:(half + 1) * NI, :] = res.results[c]["y"]
    if _trace:
        return out, res
    return out
